# revision 4
# baseline (speedup 1.0000x reference)
"""Multi-head attention (B=4, S=2048, D=1024, H=16) on 8 Trainium2 NeuronCores.

Sharding: core c handles batch b = c//2 and head-group g = c%2 (8 heads each).
Wq/Wk/Wv are column-sharded per head group, Wo row-sharded; the two partial
output projections per batch are summed on the host (+ bo).

Per-core layouts (host pre-transposes so every matmul operand has its
contraction dim on SBUF partitions):
  qT/kT/vT  [D, S]      x[b].T
  wqT       [D, FL]     (Wq[g]/8).T   (1/sqrt(dk) folded in; exact, /8)
  wkT/wvT   [D, FL]
  woT       [FL, D]     Wo[:, g cols].T
  bq/bk     [FL, 1]     per-partition bias (bq scaled by 1/8)
  bv        [1, FL]     free-axis bias (broadcast across partitions on chip)
Outputs:
  attnT     [HL, S, S]  attention probs, [sk, sq] layout (host returns a
                        transposed view)
  outp      [S, D]      partial out-projection for this head group

On-chip pipeline per (half, head): scoresT tiles [sk=128, sq=1024] on PE
(K=64 matmuls) -> exp on ACT (PSUM->SBUF) -> ctx matmuls against V augmented
with a ones column (row 64 of the PSUM accumulator = softmax row sums for
free) -> reciprocal + gpsimd partition_broadcast -> DVE normalizes the exp
tiles in place -> DMA to attnT. ctx rows are normalized the same way into
SBUF and consumed by the out-projection at the end of each half.
"""

import sys

if "/opt/trn_rl_repo" not in sys.path:
    sys.path.insert(0, "/opt/trn_rl_repo")

import numpy as np

from concourse import bacc, mybir, tile
from concourse.bass_utils import run_bass_kernel_spmd

F32 = mybir.dt.float32
P = 128
NH = 512  # fp32 matmul moving-operand / PSUM-bank max free dim

D_MODEL = 1024
NUM_HEADS = 16
D_K = 64
B_FULL = 4
S_FULL = 2048
N_CORES = 8
HEADS_LOCAL = NUM_HEADS // 2  # head-group sharding: 2 groups


def build_mha_nc(S=S_FULL, DM=D_MODEL, HL=HEADS_LOCAL, DK=D_K, exp_bufs=18):
    """Build the per-core Bass program (same program runs SPMD on all cores)."""
    FL = HL * DK          # local feature dim (512)
    ST = S // P           # sk tiles (16)
    KD = DM // P          # contraction tiles over model dim (8)
    FT = FL // P          # feature tiles (4)
    HALF = S // 2         # sq processed per phase (1024)
    NH = min(512, HALF)   # fp32 matmul moving-operand max free dim
    SC = HALF // NH       # 512-chunks per half (2)
    DC = DM // NH         # out-proj N chunks (2)
    DKA = DK + 1          # V augmented with ones column

    nc = bacc.Bacc("TRN2", target_bir_lowering=False, debug=False,
                   num_devices=N_CORES)

    qT_d = nc.declare_dram_parameter("qT", [DM, S], F32, isOutput=False)
    kT_d = nc.declare_dram_parameter("kT", [DM, S], F32, isOutput=False)
    vT_d = nc.declare_dram_parameter("vT", [DM, S], F32, isOutput=False)
    wqT_d = nc.declare_dram_parameter("wqT", [DM, FL], F32, isOutput=False)
    wkT_d = nc.declare_dram_parameter("wkT", [DM, FL], F32, isOutput=False)
    wvT_d = nc.declare_dram_parameter("wvT", [DM, FL], F32, isOutput=False)
    woT_d = nc.declare_dram_parameter("woT", [FL, DM], F32, isOutput=False)
    bq_d = nc.declare_dram_parameter("bq", [FL, 1], F32, isOutput=False)
    bk_d = nc.declare_dram_parameter("bk", [FL, 1], F32, isOutput=False)
    bv_d = nc.declare_dram_parameter("bv", [1, FL], F32, isOutput=False)
    attnT_d = nc.declare_dram_parameter("attnT", [HL, S, S], F32, isOutput=True)
    outp_d = nc.declare_dram_parameter("outp", [S, DM], F32, isOutput=True)

    with tile.TileContext(nc) as tc:
        with tc.tile_pool(name="persist", bufs=1) as pp:
            # K^T, V (ones-augmented), and biases stay resident all kernel.
            kt_sb = [pp.tile([P, S], F32, tag=f"kt{t}", name=f"kt{t}") for t in range(FT)]
            v_sb = [pp.tile([P, HL * DKA], F32, tag=f"v{t}", name=f"v{t}") for t in range(ST)]
            bq_sb = pp.tile([P, FT], F32, tag="bq")
            bk_sb = pp.tile([P, FT], F32, tag="bk")
            bv_sb = pp.tile([P, FL], F32, tag="bv")

            for ft in range(FT):
                nc.sync.dma_start(bq_sb[:, ft:ft + 1], bq_d[ft * P:(ft + 1) * P, :])
                nc.sync.dma_start(bk_sb[:, ft:ft + 1], bk_d[ft * P:(ft + 1) * P, :])
            bv_row = pp.tile([1, FL], F32, tag="bv_row")
            nc.sync.dma_start(bv_row[:], bv_d[:])
            nc.gpsimd.partition_broadcast(bv_sb[:], bv_row[:])

            # ---- K projection: kt_sb[ft] = (k[b] @ Wk_g.T + bk).T ----
            with tc.tile_pool(name="kproj", bufs=1) as ip, \
                 tc.tile_pool(name="kproj_ps", bufs=4, space="PSUM") as ps_pool:
                x_in = [ip.tile([P, S], F32, tag=f"x{kk}", name=f"x{kk}") for kk in range(KD)]
                w_in = [ip.tile([P, FL], F32, tag=f"w{kk}", name=f"w{kk}") for kk in range(KD)]
                for kk in range(KD):
                    nc.sync.dma_start(x_in[kk][:], kT_d[kk * P:(kk + 1) * P, :])
                    nc.sync.dma_start(w_in[kk][:], wkT_d[kk * P:(kk + 1) * P, :])
                for ft in range(FT):
                    for sc in range(S // NH):
                        ps = ps_pool.tile([P, NH], F32, tag="ps", bufs=4)
                        for kk in range(KD):
                            nc.tensor.matmul(
                                ps[:],
                                w_in[kk][:, ft * P:(ft + 1) * P],
                                x_in[kk][:, sc * NH:(sc + 1) * NH],
                                start=(kk == 0), stop=(kk == KD - 1))
                        nc.vector.tensor_scalar_add(
                            kt_sb[ft][:, sc * NH:(sc + 1) * NH], ps[:],
                            bk_sb[:, ft:ft + 1])

            # ---- V projection into ones-augmented per-head layout ----
            with tc.tile_pool(name="vproj", bufs=1) as ip, \
                 tc.tile_pool(name="vproj_ps", bufs=4, space="PSUM") as ps_pool:
                x_in = [ip.tile([P, S], F32, tag=f"x{kk}", name=f"x{kk}") for kk in range(KD)]
                w_in = [ip.tile([P, FL], F32, tag=f"w{kk}", name=f"w{kk}") for kk in range(KD)]
                for kk in range(KD):
                    nc.sync.dma_start(x_in[kk][:], vT_d[kk * P:(kk + 1) * P, :])
                    nc.sync.dma_start(w_in[kk][:], wvT_d[kk * P:(kk + 1) * P, :])
                for st in range(ST):
                    ps = ps_pool.tile([P, FL], F32, tag="ps", bufs=4)
                    for kk in range(KD):
                        nc.tensor.matmul(
                            ps[:],
                            x_in[kk][:, st * P:(st + 1) * P],
                            w_in[kk][:],
                            start=(kk == 0), stop=(kk == KD - 1))
                    v3 = v_sb[st].rearrange("p (h d) -> p h d", d=DKA)
                    nc.gpsimd.memset(v3[:, :, DK:DKA], 1.0)
                    nc.vector.tensor_add(
                        v3[:, :, 0:DK],
                        ps[:].rearrange("p (h d) -> p h d", d=DK),
                        bv_sb[:].rearrange("p (h d) -> p h d", d=DK))

            # ---- per-half: Q projection, attention, out-projection ----
            for half in range(2):
                c0 = half * HALF
                with tc.tile_pool(name=f"qt{half}", bufs=1) as qt_pool:
                    qt_sb = [qt_pool.tile([P, HALF], F32, tag=f"qt{t}", name=f"qt{t}")
                             for t in range(FT)]
                    with tc.tile_pool(name=f"qproj{half}", bufs=1) as ip, \
                         tc.tile_pool(name=f"qproj_ps{half}", bufs=4,
                                      space="PSUM") as ps_pool:
                        x_in = [ip.tile([P, HALF], F32, tag=f"x{kk}", name=f"x{kk}")
                                for kk in range(KD)]
                        w_in = [ip.tile([P, FL], F32, tag=f"w{kk}", name=f"w{kk}")
                                for kk in range(KD)]
                        for kk in range(KD):
                            nc.sync.dma_start(
                                x_in[kk][:], qT_d[kk * P:(kk + 1) * P, c0:c0 + HALF])
                            nc.sync.dma_start(
                                w_in[kk][:], wqT_d[kk * P:(kk + 1) * P, :])
                        for ft in range(FT):
                            for sc in range(SC):
                                ps = ps_pool.tile([P, NH], F32, tag="ps", bufs=4)
                                for kk in range(KD):
                                    nc.tensor.matmul(
                                        ps[:],
                                        w_in[kk][:, ft * P:(ft + 1) * P],
                                        x_in[kk][:, sc * NH:(sc + 1) * NH],
                                        start=(kk == 0), stop=(kk == KD - 1))
                                nc.vector.tensor_scalar_add(
                                    qt_sb[ft][:, sc * NH:(sc + 1) * NH], ps[:],
                                    bq_sb[:, ft:ft + 1])

                    with tc.tile_pool(name=f"ctxn{half}", bufs=1) as ctxn_pool:
                        ctxn_sb = [ctxn_pool.tile([P, HALF], F32, tag=f"cx{t}", name=f"cx{t}")
                                   for t in range(FT)]
                        with tc.tile_pool(name=f"attn{half}", bufs=1) as ap, \
                             tc.tile_pool(name=f"sc_ps{half}", bufs=2,
                                          space="PSUM") as sc_ps_pool, \
                             tc.tile_pool(name=f"cx_ps{half}", bufs=2,
                                          space="PSUM") as cx_ps_pool:
                            for h in range(HL):
                                hb = (h % 2) * DK   # partition base within ft tile
                                ft = h // 2
                                exp_t = []
                                for st in range(ST):
                                    sc_ps = sc_ps_pool.tile([P, HALF], F32,
                                                            tag="sc", bufs=2)
                                    for j in range(SC):
                                        nc.tensor.matmul(
                                            sc_ps[:, j * NH:(j + 1) * NH],
                                            kt_sb[ft][hb:hb + DK,
                                                      st * P:(st + 1) * P],
                                            qt_sb[ft][hb:hb + DK,
                                                      j * NH:(j + 1) * NH],
                                            start=True, stop=True)
                                    e = ap.tile([P, HALF], F32, tag="exp",
                                                bufs=exp_bufs)
                                    nc.scalar.activation(
                                        e[:], sc_ps[:],
                                        mybir.ActivationFunctionType.Exp)
                                    exp_t.append(e)
                                cx_ps = cx_ps_pool.tile([DKA, HALF], F32,
                                                        tag="cx", bufs=2)
                                for st in range(ST):
                                    v3 = v_sb[st].rearrange(
                                        "p (h d) -> p h d", d=DKA)
                                    for j in range(SC):
                                        nc.tensor.matmul(
                                            cx_ps[:, j * NH:(j + 1) * NH],
                                            v3[:, h, :],
                                            exp_t[st][:, j * NH:(j + 1) * NH],
                                            start=(st == 0), stop=(st == ST - 1))
                                rc = ap.tile([1, HALF], F32, tag="rc", bufs=2)
                                nc.vector.reciprocal(rc[:], cx_ps[DK:DKA, :])
                                rb = ap.tile([P, HALF], F32, tag="rb", bufs=2)
                                nc.gpsimd.partition_broadcast(rb[:], rc[:])
                                nc.vector.tensor_mul(
                                    ctxn_sb[ft][hb:hb + DK, :],
                                    cx_ps[0:DK, :], rb[0:DK, :])
                                for st in range(ST):
                                    nc.vector.tensor_mul(
                                        exp_t[st][:], exp_t[st][:], rb[:])
                                    nc.sync.dma_start(
                                        attnT_d[h, st * P:(st + 1) * P,
                                                c0:c0 + HALF],
                                        exp_t[st][:])

                        # ---- out-projection for this half's rows ----
                        with tc.tile_pool(name=f"oproj{half}", bufs=1) as op, \
                             tc.tile_pool(name=f"oproj_ps{half}", bufs=4,
                                          space="PSUM") as ps_pool:
                            wo_in = [op.tile([P, DM], F32, tag=f"wo{t}", name=f"wo{t}")
                                     for t in range(FT)]
                            for t in range(FT):
                                nc.sync.dma_start(
                                    wo_in[t][:], woT_d[t * P:(t + 1) * P, :])
                            for stl in range(HALF // P):
                                for dc in range(DC):
                                    ps = ps_pool.tile([P, NH], F32,
                                                      tag="ps", bufs=4)
                                    for t in range(FT):
                                        nc.tensor.matmul(
                                            ps[:],
                                            ctxn_sb[t][:, stl * P:(stl + 1) * P],
                                            wo_in[t][:, dc * NH:(dc + 1) * NH],
                                            start=(t == 0), stop=(t == FT - 1))
                                    stg = op.tile([P, NH], F32, tag="stg", bufs=3)
                                    nc.vector.tensor_copy(stg[:], ps[:])
                                    nc.sync.dma_start(
                                        outp_d[c0 + stl * P:c0 + (stl + 1) * P,
                                               dc * NH:(dc + 1) * NH],
                                        stg[:])
    nc.compile()
    return nc


def make_in_maps(q, k, v, Wq, bq, Wk, bk, Wv, bv, Wo):
    """Shard + pre-transpose full inputs into per-core input maps."""
    FL = Wq.shape[0] // 2
    in_maps = []
    for c in range(N_CORES):
        b, g = divmod(c, 2)
        sl = slice(g * FL, (g + 1) * FL)
        in_maps.append({
            "qT": np.ascontiguousarray(q[b].T),
            "kT": np.ascontiguousarray(k[b].T),
            "vT": np.ascontiguousarray(v[b].T),
            "wqT": np.ascontiguousarray(Wq[sl].T) * np.float32(0.125),
            "wkT": np.ascontiguousarray(Wk[sl].T),
            "wvT": np.ascontiguousarray(Wv[sl].T),
            "woT": np.ascontiguousarray(Wo[:, sl].T),
            "bq": (bq[sl] * np.float32(0.125)).reshape(-1, 1).copy(),
            "bk": bk[sl].reshape(-1, 1).copy(),
            "bv": bv[sl].reshape(1, -1).copy(),
        })
    return in_maps


def assemble_outputs(results, bo, B=B_FULL, S=S_FULL, DM=D_MODEL, H=NUM_HEADS):
    HL = H // 2
    out = np.empty((B, S, DM), np.float32)
    attnT = np.empty((B, H, S, S), np.float32)  # [b, h, sk, sq]
    for c in range(N_CORES):
        b, g = divmod(c, 2)
        attnT[b, g * HL:(g + 1) * HL] = results[c]["attnT"]
        if g == 0:
            out[b] = results[c]["outp"]
        else:
            out[b] += results[c]["outp"]
    out += bo.astype(np.float32)
    attn = attnT.transpose(0, 1, 3, 2)  # view: [b, h, sq, sk]
    return out, attn


_NC_CACHE = {}


def _get_nc():
    if "nc" not in _NC_CACHE:
        _NC_CACHE["nc"] = build_mha_nc()
    return _NC_CACHE["nc"]


def kernel(q, k, v, Wq, bq, Wk, bk, Wv, bv, Wo, bo):
    q = np.asarray(q, np.float32)
    k = np.asarray(k, np.float32)
    v = np.asarray(v, np.float32)
    nc = _get_nc()
    in_maps = make_in_maps(q, k, v, Wq, bq, Wk, bk, Wv, bv, Wo)
    res = run_bass_kernel_spmd(nc, in_maps, list(range(N_CORES)))
    return assemble_outputs(res.results, np.asarray(bo, np.float32))


# revision 7
# speedup vs baseline: 1.5366x; 1.5366x over previous
"""Multi-head attention (B=4, S=2048, D=1024, H=16) on 8 Trainium2 NeuronCores.

Sharding: core c handles batch b = c//2 and head-group g = c%2 (8 heads each).
Wq/Wk/Wv are column-sharded per head group, Wo row-sharded; the two partial
output projections per batch are summed on the host (+ bo).

Per-core layouts (host pre-transposes so every matmul operand has its
contraction dim on SBUF partitions):
  qT/kT/vT  [D, S]      x[b].T
  wqT       [D, FL]     (Wq[g]/8).T   (1/sqrt(dk) folded in; exact, /8)
  wkT/wvT   [D, FL]
  woT       [FL, D]     Wo[:, g cols].T
  bq/bk     [FL, 1]     per-partition bias (bq scaled by 1/8)
  bv        [1, FL]     free-axis bias (broadcast across partitions on chip)
Outputs:
  attnT     [HL, S, S]  attention probs, [sk, sq] layout (host returns a
                        transposed view)
  outp      [S, D]      partial out-projection for this head group

On-chip pipeline per (half, head): scoresT tiles [sk=128, sq=1024] on PE
(K=64 matmuls) -> exp on ACT (PSUM->SBUF) -> ctx matmuls against V augmented
with a ones column (row 64 of the PSUM accumulator = softmax row sums for
free) -> reciprocal + gpsimd partition_broadcast -> DVE normalizes the exp
tiles in place -> DMA to attnT. ctx rows are normalized the same way into
SBUF and consumed by the out-projection at the end of each half.
"""

import sys

if "/opt/trn_rl_repo" not in sys.path:
    sys.path.insert(0, "/opt/trn_rl_repo")

import numpy as np

from concourse import bacc, mybir, tile
from concourse.bass_utils import run_bass_kernel_spmd

F32 = mybir.dt.float32
P = 128
NH = 512  # fp32 matmul moving-operand / PSUM-bank max free dim

D_MODEL = 1024
NUM_HEADS = 16
D_K = 64
B_FULL = 4
S_FULL = 2048
N_CORES = 8
HEADS_LOCAL = NUM_HEADS // 2  # head-group sharding: 2 groups


def build_mha_nc(S=S_FULL, DM=D_MODEL, HL=HEADS_LOCAL, DK=D_K, exp_bufs=18,
                 use_f32r=True, norm_on_gpsimd=8):
    """Build the per-core Bass program (same program runs SPMD on all cores)."""
    FL = HL * DK          # local feature dim (512)
    ST = S // P           # sk tiles (16)
    KD = DM // P          # contraction tiles over model dim (8)
    FT = FL // P          # feature tiles (4)
    HALF = S // 2         # sq processed per phase (1024)
    NH = min(512, HALF)   # fp32 matmul moving-operand max free dim
    SC = HALF // NH       # 512-chunks per half (2)
    DC = DM // NH         # out-proj N chunks (2)
    DKA = DK + 1          # V augmented with ones column

    MDT = mybir.dt.float32r if use_f32r else F32  # matmul operand dtype

    nc = bacc.Bacc("TRN2", target_bir_lowering=False, debug=False,
                   num_devices=N_CORES)

    qT_d = nc.declare_dram_parameter("qT", [DM, S], MDT, isOutput=False)
    kT_d = nc.declare_dram_parameter("kT", [DM, S], MDT, isOutput=False)
    vT_d = nc.declare_dram_parameter("vT", [DM, S], MDT, isOutput=False)
    wqT_d = nc.declare_dram_parameter("wqT", [DM, FL], MDT, isOutput=False)
    wkT_d = nc.declare_dram_parameter("wkT", [DM, FL], MDT, isOutput=False)
    wvT_d = nc.declare_dram_parameter("wvT", [DM, FL], MDT, isOutput=False)
    woT_d = nc.declare_dram_parameter("woT", [FL, DM], MDT, isOutput=False)
    bq_d = nc.declare_dram_parameter("bq", [FL, 1], F32, isOutput=False)
    bk_d = nc.declare_dram_parameter("bk", [FL, 1], F32, isOutput=False)
    bv_d = nc.declare_dram_parameter("bv", [1, FL], F32, isOutput=False)
    attnT_d = nc.declare_dram_parameter("attnT", [HL, S, S], F32, isOutput=True)
    outp_d = nc.declare_dram_parameter("outp", [S, DM], F32, isOutput=True)

    with tile.TileContext(nc) as tc:
        with tc.tile_pool(name="persist", bufs=1) as pp:
            # K^T, V (ones-augmented), and biases stay resident all kernel.
            kt_sb = [pp.tile([P, S], MDT, tag=f"kt{t}", name=f"kt{t}") for t in range(FT)]
            v_sb = [pp.tile([P, HL * DKA], MDT, tag=f"v{t}", name=f"v{t}") for t in range(ST)]
            bq_sb = pp.tile([P, FT], F32, tag="bq")
            bk_sb = pp.tile([P, FT], F32, tag="bk")
            bv_sb = pp.tile([P, FL], F32, tag="bv")

            for ft in range(FT):
                nc.sync.dma_start(bq_sb[:, ft:ft + 1], bq_d[ft * P:(ft + 1) * P, :])
                nc.sync.dma_start(bk_sb[:, ft:ft + 1], bk_d[ft * P:(ft + 1) * P, :])
            bv_row = pp.tile([1, FL], F32, tag="bv_row")
            nc.sync.dma_start(bv_row[:], bv_d[:])
            nc.gpsimd.partition_broadcast(bv_sb[:], bv_row[:])

            # ---- K projection: kt_sb[ft] = (k[b] @ Wk_g.T + bk).T ----
            with tc.tile_pool(name="kproj", bufs=1) as ip, \
                 tc.tile_pool(name="kproj_ps", bufs=4, space="PSUM") as ps_pool:
                x_in = [ip.tile([P, S], MDT, tag=f"x{kk}", name=f"x{kk}") for kk in range(KD)]
                w_in = [ip.tile([P, FL], MDT, tag=f"w{kk}", name=f"w{kk}") for kk in range(KD)]
                for kk in range(KD):
                    nc.sync.dma_start(x_in[kk][:], kT_d[kk * P:(kk + 1) * P, :])
                    nc.sync.dma_start(w_in[kk][:], wkT_d[kk * P:(kk + 1) * P, :])
                for ft in range(FT):
                    for sc in range(S // NH):
                        ps = ps_pool.tile([P, NH], F32, tag="ps", bufs=4)
                        for kk in range(KD):
                            nc.tensor.matmul(
                                ps[:],
                                w_in[kk][:, ft * P:(ft + 1) * P],
                                x_in[kk][:, sc * NH:(sc + 1) * NH],
                                start=(kk == 0), stop=(kk == KD - 1))
                        nc.vector.tensor_scalar_add(
                            kt_sb[ft][:, sc * NH:(sc + 1) * NH], ps[:],
                            bk_sb[:, ft:ft + 1])

            # ---- V projection into ones-augmented per-head layout ----
            with tc.tile_pool(name="vproj", bufs=1) as ip, \
                 tc.tile_pool(name="vproj_ps", bufs=4, space="PSUM") as ps_pool:
                x_in = [ip.tile([P, S], MDT, tag=f"x{kk}", name=f"x{kk}") for kk in range(KD)]
                w_in = [ip.tile([P, FL], MDT, tag=f"w{kk}", name=f"w{kk}") for kk in range(KD)]
                for kk in range(KD):
                    nc.sync.dma_start(x_in[kk][:], vT_d[kk * P:(kk + 1) * P, :])
                    nc.sync.dma_start(w_in[kk][:], wvT_d[kk * P:(kk + 1) * P, :])
                for st in range(ST):
                    ps = ps_pool.tile([P, FL], F32, tag="ps", bufs=4)
                    for kk in range(KD):
                        nc.tensor.matmul(
                            ps[:],
                            x_in[kk][:, st * P:(st + 1) * P],
                            w_in[kk][:],
                            start=(kk == 0), stop=(kk == KD - 1))
                    v3 = v_sb[st].rearrange("p (h d) -> p h d", d=DKA)
                    nc.vector.tensor_scalar(
                        v3[:, :, DK:DKA],
                        ps[:, 0:HL].rearrange("p (h o) -> p h o", o=1),
                        0.0, 1.0, mybir.AluOpType.mult, mybir.AluOpType.add)
                    nc.vector.tensor_add(
                        v3[:, :, 0:DK],
                        ps[:].rearrange("p (h d) -> p h d", d=DK),
                        bv_sb[:].rearrange("p (h d) -> p h d", d=DK))

            # ---- per-half: Q projection, attention, out-projection ----
            for half in range(2):
                c0 = half * HALF
                with tc.tile_pool(name=f"qt{half}", bufs=1) as qt_pool:
                    qt_sb = [qt_pool.tile([P, HALF], MDT, tag=f"qt{t}", name=f"qt{t}")
                             for t in range(FT)]
                    with tc.tile_pool(name=f"qproj{half}", bufs=1) as ip, \
                         tc.tile_pool(name=f"qproj_ps{half}", bufs=4,
                                      space="PSUM") as ps_pool:
                        x_in = [ip.tile([P, HALF], MDT, tag=f"x{kk}", name=f"x{kk}")
                                for kk in range(KD)]
                        w_in = [ip.tile([P, FL], MDT, tag=f"w{kk}", name=f"w{kk}")
                                for kk in range(KD)]
                        for kk in range(KD):
                            nc.sync.dma_start(
                                x_in[kk][:], qT_d[kk * P:(kk + 1) * P, c0:c0 + HALF])
                            nc.sync.dma_start(
                                w_in[kk][:], wqT_d[kk * P:(kk + 1) * P, :])
                        for ft in range(FT):
                            for sc in range(SC):
                                ps = ps_pool.tile([P, NH], F32, tag="ps", bufs=4)
                                for kk in range(KD):
                                    nc.tensor.matmul(
                                        ps[:],
                                        w_in[kk][:, ft * P:(ft + 1) * P],
                                        x_in[kk][:, sc * NH:(sc + 1) * NH],
                                        start=(kk == 0), stop=(kk == KD - 1))
                                nc.vector.tensor_scalar_add(
                                    qt_sb[ft][:, sc * NH:(sc + 1) * NH], ps[:],
                                    bq_sb[:, ft:ft + 1])

                    with tc.tile_pool(name=f"ctxn{half}", bufs=1) as ctxn_pool:
                        ctxn_sb = [ctxn_pool.tile([P, HALF], MDT, tag=f"cx{t}", name=f"cx{t}")
                                   for t in range(FT)]
                        with tc.tile_pool(name=f"attn{half}", bufs=1) as ap, \
                             tc.tile_pool(name=f"sc_ps{half}", bufs=2,
                                          space="PSUM") as sc_ps_pool, \
                             tc.tile_pool(name=f"cx_ps{half}", bufs=2,
                                          space="PSUM") as cx_ps_pool:
                            for h in range(HL):
                                hb = (h % 2) * DK   # partition base within ft tile
                                ft = h // 2
                                exp_t = []
                                for st in range(ST):
                                    sc_ps = sc_ps_pool.tile([P, HALF], F32,
                                                            tag="sc", bufs=2)
                                    for j in range(SC):
                                        nc.tensor.matmul(
                                            sc_ps[:, j * NH:(j + 1) * NH],
                                            kt_sb[ft][hb:hb + DK,
                                                      st * P:(st + 1) * P],
                                            qt_sb[ft][hb:hb + DK,
                                                      j * NH:(j + 1) * NH],
                                            start=True, stop=True)
                                    e = ap.tile([P, HALF], MDT, tag="exp",
                                                bufs=exp_bufs)
                                    nc.scalar.activation(
                                        e[:], sc_ps[:],
                                        mybir.ActivationFunctionType.Exp)
                                    exp_t.append(e)
                                cx_ps = cx_ps_pool.tile([DKA, HALF], F32,
                                                        tag="cx", bufs=2)
                                for st in range(ST):
                                    v3 = v_sb[st].rearrange(
                                        "p (h d) -> p h d", d=DKA)
                                    for j in range(SC):
                                        nc.tensor.matmul(
                                            cx_ps[:, j * NH:(j + 1) * NH],
                                            v3[:, h, :],
                                            exp_t[st][:, j * NH:(j + 1) * NH],
                                            start=(st == 0), stop=(st == ST - 1))
                                rc = ap.tile([1, HALF], F32, tag="rc", bufs=2)
                                nc.vector.reciprocal(rc[:], cx_ps[DK:DKA, :])
                                rb = ap.tile([P, HALF], F32, tag="rb", bufs=2)
                                nc.gpsimd.partition_broadcast(rb[:], rc[:])
                                nc.vector.tensor_mul(
                                    ctxn_sb[ft][hb:hb + DK, :],
                                    cx_ps[0:DK, :], rb[0:DK, :])
                                for st in range(ST):
                                    e = exp_t[st][:]
                                    eng = (nc.gpsimd
                                           if st % ST < norm_on_gpsimd * ST // 16
                                           else nc.vector)
                                    eng.tensor_mul(e, e, rb[:])
                                    nc.sync.dma_start(
                                        attnT_d[h, st * P:(st + 1) * P,
                                                c0:c0 + HALF],
                                        e.bitcast(F32))

                        # ---- out-projection for this half's rows ----
                        with tc.tile_pool(name=f"oproj{half}", bufs=1) as op, \
                             tc.tile_pool(name=f"oproj_ps{half}", bufs=4,
                                          space="PSUM") as ps_pool:
                            wo_in = [op.tile([P, DM], MDT, tag=f"wo{t}", name=f"wo{t}")
                                     for t in range(FT)]
                            for t in range(FT):
                                nc.sync.dma_start(
                                    wo_in[t][:], woT_d[t * P:(t + 1) * P, :])
                            for stl in range(HALF // P):
                                for dc in range(DC):
                                    ps = ps_pool.tile([P, NH], F32,
                                                      tag="ps", bufs=4)
                                    for t in range(FT):
                                        nc.tensor.matmul(
                                            ps[:],
                                            ctxn_sb[t][:, stl * P:(stl + 1) * P],
                                            wo_in[t][:, dc * NH:(dc + 1) * NH],
                                            start=(t == 0), stop=(t == FT - 1))
                                    stg = op.tile([P, NH], F32, tag="stg", bufs=3)
                                    nc.vector.tensor_copy(stg[:], ps[:])
                                    nc.sync.dma_start(
                                        outp_d[c0 + stl * P:c0 + (stl + 1) * P,
                                               dc * NH:(dc + 1) * NH],
                                        stg[:])
    nc.compile()
    return nc


def make_in_maps(q, k, v, Wq, bq, Wk, bk, Wv, bv, Wo):
    """Shard + pre-transpose full inputs into per-core input maps."""
    FL = Wq.shape[0] // 2
    in_maps = []
    for c in range(N_CORES):
        b, g = divmod(c, 2)
        sl = slice(g * FL, (g + 1) * FL)
        in_maps.append({
            "qT": np.ascontiguousarray(q[b].T),
            "kT": np.ascontiguousarray(k[b].T),
            "vT": np.ascontiguousarray(v[b].T),
            "wqT": np.ascontiguousarray(Wq[sl].T) * np.float32(0.125),
            "wkT": np.ascontiguousarray(Wk[sl].T),
            "wvT": np.ascontiguousarray(Wv[sl].T),
            "woT": np.ascontiguousarray(Wo[:, sl].T),
            "bq": (bq[sl] * np.float32(0.125)).reshape(-1, 1).copy(),
            "bk": bk[sl].reshape(-1, 1).copy(),
            "bv": bv[sl].reshape(1, -1).copy(),
        })
    return in_maps


def assemble_outputs(results, bo, B=B_FULL, S=S_FULL, DM=D_MODEL, H=NUM_HEADS):
    HL = H // 2
    out = np.empty((B, S, DM), np.float32)
    attnT = np.empty((B, H, S, S), np.float32)  # [b, h, sk, sq]
    for c in range(N_CORES):
        b, g = divmod(c, 2)
        attnT[b, g * HL:(g + 1) * HL] = results[c]["attnT"]
        if g == 0:
            out[b] = results[c]["outp"]
        else:
            out[b] += results[c]["outp"]
    out += bo.astype(np.float32)
    attn = attnT.transpose(0, 1, 3, 2)  # view: [b, h, sq, sk]
    return out, attn


_NC_CACHE = {}


def _get_nc():
    if "nc" not in _NC_CACHE:
        _NC_CACHE["nc"] = build_mha_nc()
    return _NC_CACHE["nc"]


def kernel(q, k, v, Wq, bq, Wk, bk, Wv, bv, Wo, bo):
    q = np.asarray(q, np.float32)
    k = np.asarray(k, np.float32)
    v = np.asarray(v, np.float32)
    nc = _get_nc()
    in_maps = make_in_maps(q, k, v, Wq, bq, Wk, bk, Wv, bv, Wo)
    res = run_bass_kernel_spmd(nc, in_maps, list(range(N_CORES)))
    return assemble_outputs(res.results, np.asarray(bo, np.float32))
